# revision 1
# baseline (speedup 1.0000x reference)
"""MDCT (conv1d stride-512, kernel-1024, pad-512) as a Bass/Tile kernel on 8 trn2 cores.

Strategy
--------
out[b,k,j] = sum_t F[k,t] * xpad[b, j*512 + t],  x:[16,1,1048576] -> out:[16,512,2049]

* Data-parallel over batch: 2 batches per NeuronCore (8 cores).
* MDCT fold: the 2N=1024 window folds into an N=512 DCT-IV, halving matmul K:
    frame j window = [A_{j-1}, A_j]  (A_j = x[j*512:(j+1)*512])
    g2[u] = A[255-u] + A[256+u]  (u in [0,256), computed per block A)
    g1[v] = A[v]     - A[511-v]
    out[:,j] = -C'[:,0:256] @ g2(A_j) + C'[:,256:512] @ g1(A_{j-1})
  where C'[k,u] is extracted from the filter itself (least-squares over the two
  redundant copies of each coefficient present in F).
* On-chip: load x blocks in natural layout [block, sample] (2KB-contiguous DMA),
  fold on DVE (negative-stride reads), transpose [block,u]->[u,block] on the PE
  (identity transpose), matmul in float32r (TF32-like, 4x faster than fp32).
"""

import numpy as np

N = 512
B = 16
T = 2048
NCORES = 8
BPC = B // NCORES          # batches per core = 2
JCHUNK = 512               # frames per chunk (PSUM bank = 512 fp32)
NCHUNK = T // JCHUNK       # 4 full chunks; frame 2048 handled as tail
SAMP = N * T               # samples per batch

_compiled = None


def _build():
    import concourse.bass as bass
    import concourse.mybir as mybir
    from concourse import bacc
    from concourse.tile import TileContext
    from concourse.masks import make_identity

    f32 = mybir.dt.float32
    f32r = mybir.dt.float32r

    nc = bacc.Bacc("TRN2", target_bir_lowering=False, debug=False)

    xs_d = nc.dram_tensor("xs", [BPC, SAMP], f32, kind="ExternalInput").ap()
    w_d = nc.dram_tensor("wt", [4, 128, N], f32, kind="ExternalInput").ap()
    o_d = nc.dram_tensor("os", [BPC, N, T + 1], f32, kind="ExternalOutput").ap()

    with TileContext(nc) as tc:
        with tc.tile_pool(name="wp", bufs=1) as wp, \
             tc.tile_pool(name="xp", bufs=8) as xp, \
             tc.tile_pool(name="gp", bufs=8) as gp, \
             tc.tile_pool(name="mtp", bufs=2) as mtp, \
             tc.tile_pool(name="op", bufs=8) as op, \
             tc.tile_pool(name="tps", bufs=4, space="PSUM") as tps, \
             tc.tile_pool(name="ops", bufs=4, space="PSUM") as ops:

            ident = wp.tile([128, 128], f32, tag="ident")
            make_identity(nc, ident[:])
            z0 = wp.tile([128, 1], f32, tag="z0")
            nc.vector.memset(z0[:], 0.0)
            ident_r = wp.tile([128, 128], f32r, tag="identr")
            nc.vector.tensor_copy(out=ident_r[:], in_=ident[:])

            W = []
            for uc in range(4):
                w_t = wp.tile([128, N], f32r, tag=f"w{uc}")
                nc.gpsimd.dma_start(out=w_t[:], in_=w_d[uc])  # cast fp32 -> fp32r
                W.append(w_t)

            for b in range(BPC):
                prev_mt = None  # previous chunk's MT tiles (for col 0 = block j0-1)
                for jc in range(NCHUNK):
                    j0 = jc * JCHUNK
                    # ---- load 4 natural-layout tiles [128 blocks, 512 samples]
                    X = []
                    for t in range(4):
                        x_t = xp.tile([128, N], f32, tag="x")
                        s0 = (j0 + 128 * t) * N
                        nc.sync.dma_start(
                            out=x_t[:],
                            in_=xs_d[b, s0:s0 + 128 * N].rearrange("(p f) -> p f", p=128),
                        )
                        X.append(x_t)
                    # ---- fold on DVE -> G tiles [128 blocks, 512 u]
                    G = []
                    for t in range(4):
                        g_t = gp.tile([128, N], f32r, tag="g")
                        nc.vector.tensor_add(g_t[:, 0:256], X[t][:, 255::-1], X[t][:, 256:512])
                        nc.vector.tensor_sub(g_t[:, 256:512], X[t][:, 0:256], X[t][:, 511:255:-1])
                        G.append(g_t)
                    # ---- MT tiles [128 u, 513 blockcols] per u-chunk, fp32r
                    MT = [mtp.tile([128, JCHUNK + 1], f32r, tag=f"mt{uc}", name=f"mt{uc}")
                          for uc in range(4)]
                    for uc in (2, 3):
                        if jc == 0:
                            nc.vector.tensor_copy(out=MT[uc][:, 0:1], in_=z0[:])
                        else:
                            nc.vector.tensor_copy(out=MT[uc][:, 0:1], in_=prev_mt[uc][:, JCHUNK:JCHUNK + 1])
                    # ---- PE transposes [block,u] -> [u,block] + copies into MT
                    for t in range(4):
                        for uc in range(4):
                            p_t = tps.tile([128, 128], f32r, tag="tp")
                            nc.tensor.transpose(p_t[:], G[t][:, 128 * uc:128 * (uc + 1)], ident_r[:])
                            cp = nc.scalar.copy if uc < 2 else nc.vector.tensor_copy
                            cp(out=MT[uc][:, 1 + 128 * t:129 + 128 * t], in_=p_t[:])
                    # ---- main matmuls: psum[kc] = sum_uc W[uc][:,kc].T @ MT[uc]
                    for kc in range(4):
                        po = ops.tile([128, JCHUNK], mybir.dt.float32, tag="po")
                        for uc in range(4):
                            rhs = MT[uc][:, 1:JCHUNK + 1] if uc < 2 else MT[uc][:, 0:JCHUNK]
                            nc.tensor.matmul(
                                po[:], W[uc][:, 128 * kc:128 * (kc + 1)], rhs,
                                start=(uc == 0), stop=(uc == 3),
                            )
                        o_t = op.tile([128, JCHUNK], f32, tag="o")
                        nc.scalar.copy(out=o_t[:], in_=po[:])
                        nc.sync.dma_start(
                            out=o_d[b, 128 * kc:128 * (kc + 1), j0:j0 + JCHUNK],
                            in_=o_t[:],
                        )
                    prev_mt = MT

                # ---- tail frame j=2048: out[:,2048] = C'[:,256:] @ g1(A_2047)
                ot = op.tile([128, 4], f32, tag="otail")
                for kc in range(4):
                    pt = tps.tile([128, 4], mybir.dt.float32, tag="tp", name="pt")
                    for i, uc in enumerate((2, 3)):
                        nc.tensor.matmul(
                            pt[:, 0:1],
                            W[uc][:, 128 * kc:128 * (kc + 1)].bitcast(f32),
                            prev_mt[uc][:, JCHUNK:JCHUNK + 1].bitcast(f32),
                            start=(i == 0), stop=(i == 1),
                        )
                    nc.scalar.copy(out=ot[:, kc:kc + 1], in_=pt[:, 0:1])
                nc.sync.dma_start(
                    out=o_d[b, :, T:T + 1].rearrange("(c p) o -> p (c o)", p=128),
                    in_=ot[:],
                )

    nc.compile()
    return nc


def _weights(mdct_filter: np.ndarray) -> np.ndarray:
    """Extract DCT-IV weight tiles W[4,128,512] from the 1024-tap filter.

    Each C'[k,u] coefficient appears twice in F (up to sign); average the two
    copies (least squares) to minimize the fold residual.
    """
    F = mdct_filter.reshape(N, 2 * N).astype(np.float64)
    sideA = np.concatenate([-F[:, 768:1024], F[:, 0:256]], axis=1)
    sideB = -F[:, 767:255:-1]
    Cp = 0.5 * (sideA + sideB)  # [k, u]
    W = np.empty((4, 128, N), dtype=np.float32)
    W[0] = -Cp[:, 0:128].T
    W[1] = -Cp[:, 128:256].T
    W[2] = Cp[:, 256:384].T
    W[3] = Cp[:, 384:512].T
    return W


def kernel(x: np.ndarray, mdct_filter: np.ndarray, _trace=False) -> np.ndarray:
    global _compiled
    from concourse.bass_utils import run_bass_kernel_spmd

    if _compiled is None:
        _compiled = _build()
    nc = _compiled

    x = np.ascontiguousarray(np.asarray(x, dtype=np.float32)).reshape(B, SAMP)
    wt = _weights(np.asarray(mdct_filter, dtype=np.float32))

    in_maps = [
        {"xs": x[c * BPC:(c + 1) * BPC], "wt": wt}
        for c in range(NCORES)
    ]
    res = run_bass_kernel_spmd(nc, in_maps, core_ids=list(range(NCORES)),
                               trace=_trace)
    out = np.empty((B, N, T + 1), dtype=np.float32)
    for c in range(NCORES):
        out[c * BPC:(c + 1) * BPC] = res.results[c]["os"]
    if _trace:
        kernel._last_results = res
    return out



# revision 4
# speedup vs baseline: 1.6679x; 1.6679x over previous
"""MDCT (conv1d stride-512, kernel-1024, pad-512) as a Bass/Tile kernel on 8 trn2 cores.

Strategy
--------
out[b,k,j] = sum_t F[k,t] * xpad[b, j*512 + t],  x:[16,1,1048576] -> out:[16,512,2049]

* Data-parallel over batch: 2 batches per NeuronCore (8 cores).
* MDCT fold halves the matmul contraction (2N=1024 window -> N=512 DCT-IV):
    frame j window = [A_{j-1}, A_j]  (A_j = x[j*512:(j+1)*512])
    g2'[q] = A[q] + A[511-q]   (q in [0,256))   [= g2 reversed]
    g1 [q] = A[q] - A[511-q]
    out[:,j] = W2' @ g2'(A_j) + W1 @ g1(A_{j-1})
  where the weight matrices come from the filter itself (least-squares over the
  two redundant copies of each coefficient present in F), with the u-reversal
  of g2 absorbed into a host-side weight column permutation.
* Transpose-free: the host delivers x as two bf16 planes xp0[q,j]=A_j[q],
  xp1[q,j]=A_j[511-q] (a pure layout permutation), so the fold is a plain
  partition-aligned add/sub on the DVE and its outputs land directly in
  [contraction, frame] layout as matmul moving operands. No PE transposes,
  no PSUM staging of the rhs.
* bf16 end-to-end (inputs, weights, outputs) with fp32 PSUM accumulation;
  host upcasts the result to fp32.
"""

import numpy as np

N = 512
B = 16
T = 2048
NCORES = 8
BPC = B // NCORES          # batches per core = 2
JCHUNK = 512               # frames per chunk (PSUM bank = 512 fp32)
NCHUNK = T // JCHUNK       # 4 full chunks; frame 2048 handled as tail

_compiled = None


def _build():
    import concourse.mybir as mybir
    from concourse import bacc
    from concourse.tile import TileContext

    f32 = mybir.dt.float32
    bf16 = mybir.dt.bfloat16

    nc = bacc.Bacc("TRN2", target_bir_lowering=False, debug=False)

    # xp[b, c, qc, p, j]: c=0 plane A_j[q], c=1 plane A_j[511-q], q = 128*qc+p
    xp_d = nc.dram_tensor("xp", [BPC, 2, 2, 128, T], bf16, kind="ExternalInput").ap()
    w_d = nc.dram_tensor("wt", [4, 128, N], bf16, kind="ExternalInput").ap()
    o_d = nc.dram_tensor("os", [BPC, N, T + 1], bf16, kind="ExternalOutput").ap()

    with TileContext(nc) as tc:
        with tc.tile_pool(name="wp", bufs=1) as wp, \
             tc.tile_pool(name="xp", bufs=8) as xpool, \
             tc.tile_pool(name="g2p", bufs=6) as g2p, \
             tc.tile_pool(name="g1p", bufs=6) as g1p, \
             tc.tile_pool(name="op", bufs=4) as op, \
             tc.tile_pool(name="ops", bufs=8, space="PSUM") as ops:

            W = []
            for uc in range(4):
                w_t = wp.tile([128, N], bf16, tag=f"w{uc}")
                nc.sync.dma_start(out=w_t[:], in_=w_d[uc])
                W.append(w_t)

            for b in range(BPC):
                g1_last = None
                for jc in range(NCHUNK):
                    j0 = jc * JCHUNK
                    # ---- load pair-planes [128 q, 2 c, 513 j] (cols j0-1..j0+511)
                    X = []
                    for qc in range(2):
                        x_t = xpool.tile([128, 2, JCHUNK + 1], bf16, tag=f"x{qc}")
                        if jc == 0:
                            nc.vector.memset(x_t[:, :, 0:1], 0.0)
                            nc.sync.dma_start(
                                out=x_t[:, :, 1:JCHUNK + 1],
                                in_=xp_d[b, :, qc, :, 0:JCHUNK].rearrange(
                                    "c p j -> p c j"),
                            )
                        else:
                            nc.sync.dma_start(
                                out=x_t[:],
                                in_=xp_d[b, :, qc, :, j0 - 1:j0 + JCHUNK].rearrange(
                                    "c p j -> p c j"),
                            )
                        X.append(x_t)
                    # ---- fold on DVE: g2' = p0+p1 (frames j0..), g1 = p0-p1 (shifted)
                    w1 = JCHUNK + 1 if jc == NCHUNK - 1 else JCHUNK
                    G2, G1 = [], []
                    for qc in range(2):
                        g2_t = g2p.tile([128, JCHUNK], bf16, tag=f"g2{qc}")
                        nc.vector.tensor_add(
                            g2_t[:],
                            X[qc][:, 0, 1:JCHUNK + 1], X[qc][:, 1, 1:JCHUNK + 1])
                        g1_t = g1p.tile([128, JCHUNK + 1], bf16, tag=f"g1{qc}")
                        nc.vector.tensor_sub(
                            g1_t[:, 0:w1],
                            X[qc][:, 0, 0:w1], X[qc][:, 1, 0:w1])
                        G2.append(g2_t)
                        G1.append(g1_t)
                    # ---- matmuls: po = W0@g2'lo + W1@g2'hi + W2@g1lo + W3@g1hi
                    ot = op.tile([128, 4, JCHUNK], bf16, tag="o")
                    for kc in range(4):
                        po = ops.tile([128, JCHUNK], f32, tag="po")
                        ks = slice(128 * kc, 128 * (kc + 1))
                        nc.tensor.matmul(po[:], W[0][:, ks], G2[0][:],
                                         start=True, stop=False)
                        nc.tensor.matmul(po[:], W[1][:, ks], G2[1][:],
                                         start=False, stop=False)
                        nc.tensor.matmul(po[:], W[2][:, ks], G1[0][:, 0:JCHUNK],
                                         start=False, stop=False)
                        nc.tensor.matmul(po[:], W[3][:, ks], G1[1][:, 0:JCHUNK],
                                         start=False, stop=True)
                        cp = nc.scalar.copy if kc % 2 == 0 else nc.vector.tensor_copy
                        cp(out=ot[:, kc], in_=po[:])
                    nc.sync.dma_start(
                        out=o_d[b, :, j0:j0 + JCHUNK].rearrange(
                            "(c p) j -> p c j", p=128),
                        in_=ot[:],
                    )
                    g1_last = G1

                # ---- tail frame j=2048: out[:,2048] = W2@g1lo + W3@g1hi (col 512)
                otail = op.tile([128, 4], bf16, tag="otail")
                for kc in range(4):
                    pt = ops.tile([128, JCHUNK], f32, tag="po", name="pt")
                    ks = slice(128 * kc, 128 * (kc + 1))
                    nc.tensor.matmul(pt[:, 0:1], W[2][:, ks],
                                     g1_last[0][:, JCHUNK:JCHUNK + 1],
                                     start=True, stop=False)
                    nc.tensor.matmul(pt[:, 0:1], W[3][:, ks],
                                     g1_last[1][:, JCHUNK:JCHUNK + 1],
                                     start=False, stop=True)
                    nc.scalar.copy(out=otail[:, kc:kc + 1], in_=pt[:, 0:1])
                nc.sync.dma_start(
                    out=o_d[b, :, T:T + 1].rearrange("(c p) o -> p (c o)", p=128),
                    in_=otail[:],
                )

    nc.compile()
    return nc


def _weights(mdct_filter: np.ndarray) -> np.ndarray:
    """Extract DCT-IV weight tiles W[4,128,512] from the 1024-tap filter.

    Each coefficient appears twice in F (up to sign); average the two copies
    (least squares) to minimize the fold residual. Column order matches the
    on-device g2'/g1 fold layout (g2 reversed into g2').
    """
    F = mdct_filter.reshape(N, 2 * N).astype(np.float64)
    sideA = np.concatenate([-F[:, 768:1024], F[:, 0:256]], axis=1)
    sideB = -F[:, 767:255:-1]
    Cp = 0.5 * (sideA + sideB)  # [k, u]
    W = np.empty((4, 128, N), dtype=np.float64)
    W[0] = -Cp[:, 255:127:-1].T   # g2' lo: row q ↔ u = 255-q
    W[1] = -Cp[:, 127::-1].T      # g2' hi: row q ↔ u = 127-q
    W[2] = Cp[:, 256:384].T       # g1 lo
    W[3] = Cp[:, 384:512].T       # g1 hi
    return W


def kernel(x: np.ndarray, mdct_filter: np.ndarray, _trace=False) -> np.ndarray:
    global _compiled
    import ml_dtypes
    from concourse.bass_utils import run_bass_kernel_spmd

    bf16 = ml_dtypes.bfloat16
    if _compiled is None:
        _compiled = _build()
    nc = _compiled

    xr = np.ascontiguousarray(np.asarray(x, dtype=np.float32)).reshape(B, T, N)
    xp0 = xr[:, :, 0:256].transpose(0, 2, 1)             # [B, 256, T] = A_j[q]
    xp1 = xr[:, :, 256:512][:, :, ::-1].transpose(0, 2, 1)  # A_j[511-q]
    xp = np.stack([xp0, xp1], axis=1).astype(bf16).reshape(B, 2, 2, 128, T)
    wt = _weights(np.asarray(mdct_filter, dtype=np.float32)).astype(bf16)

    in_maps = [
        {"xp": xp[c * BPC:(c + 1) * BPC], "wt": wt}
        for c in range(NCORES)
    ]
    res = run_bass_kernel_spmd(nc, in_maps, core_ids=list(range(NCORES)),
                               trace=_trace)
    out = np.empty((B, N, T + 1), dtype=np.float32)
    for c in range(NCORES):
        out[c * BPC:(c + 1) * BPC] = np.asarray(
            res.results[c]["os"]).astype(np.float32)
    if _trace:
        kernel._last_results = res
    return out
